# revision 58
# baseline (speedup 1.0000x reference)
"""Trainium2 Bass kernel for NeoX-style attention block (B=2, S=2048, D=2048,
H=16, HS=128, partial RoPE rot=32, no mask) sharded over 8 NeuronCores.

Sharding: core c handles batch b = c//4 and head group g = c%4 (4 heads).
Tensor-parallel over heads: W_qkv column-sliced, W_dense row-sliced; each core
produces a partial [S, D] output (bf16); host sums 4 partials per batch and
adds (b_v @ W_dense + b_dense), which is exact because softmax rows sum to 1.

v3 design (single QKV pass, bf16 activations/weights, fp32 PSUM):
  - V computed directly in [token, hs] layout (stationary = hT chunks,
    moving = W_v columns of all 4 heads, N=512) -- no PE transposes.
  - q/k computed in [dim, token] layout (stationary = W chunks, moving = hT
    windows); fins = DVE tensor_scalar per 512 half (fused bias+softmax
    scale) on a 2x[128,512] PSUM ring so chunk handoffs pipeline.
  - partial RoPE entirely on DVE: q/k head dims are host-permuted (rot_lo ->
    partitions 0-15, rot_hi -> 32-47, 32-aligned bases) so rotate_half is
    two cross-quadrant muls + one 48-partition cos mul (ones rows 16-31)
    + sub/add.  The PE never touches RoPE.
  - attention per head: scores S^T = K_chunk^T @ Q (fp32 PSUM, 2x N=512 per
    bank), exp on ACT (bf16 out), AV accumulation; softmax denominators via
    dual bf16 accumulator chains (Pool: even k2<12, DVE: rest) merged
    mid-loop at k2==12 so pden (2 ones-MMs) launches straight off aco15;
    copy-first normalize: po -> oT (ACT+DVE halves) frees the po bank early,
    reciprocal+in-place scale follow off-path.
  - fillers: qk(h+1) production interleaved into attn(h) (front-loaded at
    block starts), qk(0) into the V phase (wp-major, DMA-arrival paced);
    head 3 qs1 interleaves the first dense tiles (dead qk PSUM ring).
  - dense: lhsT = O^T chunks, rhs = W_dense rows (bf16, N=512); pd tiles
    alternate the pS/qk rings (depth 4) so copies never gate matmuls; the
    final tile drains via quarter copies (ACT+DVE) + 4 DMAs on 2 queues.
  - all matmul outputs are single-PSUM-bank (N<=512 fp32) -- a TRN2 hard
    constraint; wider tiles exist only for ACT/DVE/Pool ops.
"""
import sys

sys.path.insert(0, "/opt/trn_rl_repo")

import numpy as np
import ml_dtypes
from contextlib import ExitStack

import concourse.bass as bass  # noqa: F401  (registers engine types)
import concourse.tile as tile
from concourse import bacc, mybir
from concourse import bass_utils

F32 = mybir.dt.float32
F32R = mybir.dt.float32r
BF16 = mybir.dt.bfloat16
MUL = mybir.AluOpType.mult
ADD = mybir.AluOpType.add
EXP = mybir.ActivationFunctionType.Exp
IDENT = mybir.ActivationFunctionType.Identity

B, S, D = 2, 2048, 2048
H, HS, ROT = 16, 128, 32
BASE = 10000.0
SM_SCALE = 1.0 / float(np.sqrt(HS))

HPC = 4            # heads per core
CPB = 4            # cores per batch
NCORES = 8
KC = D // 128      # 16 contraction chunks
NW = 4             # token windows of 512
WIN = 512
NM = 2 * HPC       # 8 q/k m-chunks (m = 2*h + {0:q, 1:k})

_NC = None
TRACE = False
LAST_RESULT = [None]
NBF = ml_dtypes.bfloat16


def _build(repeat=1, bench=False):
    nc = bacc.Bacc("TRN2", target_bir_lowering=False, debug=False)
    # host-prearranged [128, X] layouts, contiguous per partition
    ht = nc.dram_tensor("ht", [128, NW * KC * WIN], BF16, kind="ExternalInput").ap()
    wqk = nc.dram_tensor("wqk", [128, NM * KC * 128], BF16, kind="ExternalInput").ap()
    wv = nc.dram_tensor("wv", [128, KC * 512], BF16, kind="ExternalInput").ap()
    wd = nc.dram_tensor("wd", [128, HPC * D], BF16, kind="ExternalInput").ap()
    tabc = nc.dram_tensor("tabc", [48, S], BF16, kind="ExternalInput").ap()
    tabs = nc.dram_tensor("tabs", [48, S], BF16, kind="ExternalInput").ap()
    ones = nc.dram_tensor("ones", [128, 128], BF16, kind="ExternalInput").ap()
    bqk = nc.dram_tensor("bqk", [128, NM], F32, kind="ExternalInput").ap()
    outp = nc.dram_tensor("outp", [S, D], BF16,
                          kind="Internal" if bench else "ExternalOutput").ap()
    probe = (nc.dram_tensor("probe", [128, 4], F32, kind="ExternalOutput").ap()
             if bench else None)

    with tile.TileContext(nc) as tc:
      for _rep in range(repeat):
        with ExitStack() as ctx:
            glob = ctx.enter_context(tc.tile_pool(name="glob", bufs=1))
            epool = ctx.enter_context(tc.tile_pool(name="epool", bufs=1))

            # ---- persistent activations ----
            qkT = glob.tile([128, NM * S], BF16, tag="qkT")      # q/k^T
            v_sb = glob.tile([128, KC * 512], BF16, tag="v_sb")  # V [tok, hs]
            oT = glob.tile([128, HPC * S], BF16, tag="oT")       # O^T per head

            # ---- constants / tables ----
            tabc_sb = glob.tile([48, S], BF16, tag="tabc")
            tabs_sb = glob.tile([48, S], BF16, tag="tabs")
            ones_sb = glob.tile([128, 128], BF16, tag="ones")
            bqk_sb = glob.tile([128, NM], F32, tag="bqk")

            with ExitStack() as s1:
                bigp = s1.enter_context(tc.tile_pool(name="bigp", bufs=1))
                ht_sb = bigp.tile([128, NW * KC * WIN], BF16, tag="ht")
                wqk_sb = bigp.tile([128, NM * KC * 128], BF16, tag="wqk")
                sv = ExitStack()
                wvp = sv.enter_context(tc.tile_pool(name="wvp", bufs=1))
                wv_sb = wvp.tile([128, KC * 512], BF16, tag="wv")

                # ---- input DMAs: two HWDGE queues in parallel ----
                # ACT queue: wv + qk weights + tables + ht w2;
                # SP queue: ht w0 (8ths), w1 (quarters), w3, then wqk m2-7.
                # Fine split of the first pieces so the V chain starts early
                # and the interleaved qk(0) drains never outrun the DMA.
                Q8 = KC * WIN // 8
                V8 = KC * 512 // 8
                for q in range(8):
                    nc.scalar.dma_start(wv_sb[:, q * V8:(q + 1) * V8],
                                        wv[:, q * V8:(q + 1) * V8])
                    nc.sync.dma_start(ht_sb[:, q * Q8:(q + 1) * Q8],
                                      ht[:, q * Q8:(q + 1) * Q8])
                for m in range(2):
                    for hh in range(2):
                        sl = slice((2 * m + hh) * KC * 64,
                                   (2 * m + hh + 1) * KC * 64)
                        nc.scalar.dma_start(wqk_sb[:, sl], wqk[:, sl])
                nc.scalar.dma_start(bqk_sb[:], bqk)
                nc.scalar.dma_start(tabc_sb[:], tabc)
                nc.scalar.dma_start(tabs_sb[:], tabs)
                nc.scalar.dma_start(ones_sb[:], ones)
                for q in range(8):      # w1 in 8ths (sync)
                    nc.sync.dma_start(
                        ht_sb[:, KC * WIN + q * Q8:KC * WIN + (q + 1) * Q8],
                        ht[:, KC * WIN + q * Q8:KC * WIN + (q + 1) * Q8])
                half = KC * WIN // 2
                for hh in range(2):     # w2 in halves (scalar queue)
                    nc.scalar.dma_start(
                        ht_sb[:, 2 * KC * WIN + hh * half:
                              2 * KC * WIN + (hh + 1) * half],
                        ht[:, 2 * KC * WIN + hh * half:
                           2 * KC * WIN + (hh + 1) * half])
                for hh in range(2):     # w3 in halves (sync)
                    nc.sync.dma_start(
                        ht_sb[:, 3 * KC * WIN + hh * half:
                              3 * KC * WIN + (hh + 1) * half],
                        ht[:, 3 * KC * WIN + hh * half:
                           3 * KC * WIN + (hh + 1) * half])
                for m in range(2, NM):
                    nc.sync.dma_start(
                        wqk_sb[:, m * KC * 128:(m + 1) * KC * 128],
                        wqk[:, m * KC * 128:(m + 1) * KC * 128])

                htv = ht_sb[:].rearrange("p (w kc j) -> p w kc j",
                                         w=NW, kc=KC)

                with ExitStack() as sA:
                    ps = sA.enter_context(
                        tc.tile_pool(name="ps", bufs=1, space="PSUM"))
                    # PSUM budget (8 banks): qk 1x2 + pS 2x2 + po 1x2 = 8.
                    # V-phase pv tiles ride the pS ring (before any pS tile).

                    # ---- q/k production steps (generator, interleavable).
                    # N=1024 over a window pair; RoPE runs one step deferred
                    # and entirely on DVE.  The q/k head dims are host-permuted
                    # so rot_lo sits in partitions 0-15 and rot_hi in 32-47
                    # (32-aligned bases -- a DVE requirement for cross-quadrant
                    # reads); rotate_half is then two shifted muls, a single
                    # 48-partition cos mul (rows 16-31 of the table are ones),
                    # and a subtract/add pair.  The PE never touches RoPE.
                    SUB = mybir.AluOpType.subtract
                    def rope(m, wp):
                        sl = slice(m * S + wp * 1024, m * S + (wp + 1) * 1024)
                        wsl = slice(wp * 1024, (wp + 1) * 1024)
                        # DVE TT wants equal input base partitions; only the
                        # OUTPUT may land in another quadrant.  sin lives at
                        # rows 0-15 AND 32-47 of tabs so each mul reads both
                        # inputs at one base and writes the shifted quadrant.
                        t16 = epool.tile([48, 1024], BF16, tag="t16",
                                         bufs=1, name=f"t16_{m}_{wp}")
                        nc.vector.tensor_tensor(
                            t16[0:16, :], qkT[32:48, sl],
                            tabs_sb[32:48, wsl], op=MUL)
                        nc.vector.tensor_tensor(
                            t16[32:48, :], qkT[0:16, sl],
                            tabs_sb[0:16, wsl], op=MUL)
                        nc.vector.tensor_tensor(
                            qkT[0:48, sl], qkT[0:48, sl],
                            tabc_sb[:, wsl], op=MUL)
                        nc.vector.tensor_tensor(
                            qkT[0:16, sl], qkT[0:16, sl], t16[0:16, :], op=SUB)
                        nc.vector.tensor_tensor(
                            qkT[32:48, sl], qkT[32:48, sl], t16[32:48, :],
                            op=ADD)

                    def qk_steps(h):
                        # wp-major: the wp=0 steps only touch ht windows 0-1,
                        # so the V-phase drains can start before w2/w3 land
                        pending = None
                        for wp in range(2):
                            for m in (2 * h, 2 * h + 1):
                                # two [128,512] tiles on a bufs=2 ring: the
                                # next chunk's matmuls only wait for fin of
                                # the same half, so the PE->DVE handoff
                                # pipelines across chunk boundaries
                                pq0 = ps.tile([128, 512], F32, tag="qk",
                                              bufs=2, name=f"pq{m}_{wp}a")
                                pq1 = ps.tile([128, 512], F32, tag="qk",
                                              bufs=2, name=f"pq{m}_{wp}b")
                                pqs = (pq0, pq1)

                                def mm2(kc2, m=m, wp=wp, pqs=pqs):
                                    def f():
                                        for kc in range(2 * kc2, 2 * kc2 + 2):
                                            for i in range(2):
                                                nc.tensor.matmul(
                                                    pqs[i][:],
                                                    wqk_sb[:, (m * KC + kc) * 128:
                                                           (m * KC + kc + 1) * 128],
                                                    htv[:, 2 * wp + i, kc, :],
                                                    start=(kc == 0),
                                                    stop=(kc == KC - 1))
                                    return f
                                for kc2 in range(8):
                                    yield mm2(kc2)

                                def fin(m=m, wp=wp, pqs=pqs, prev=pending):
                                    def f():
                                        # DVE, not ACT: during attention the
                                        # ACT queue is deep with exps and the
                                        # pq ring would stall behind them
                                        for i in range(2):
                                            sl = slice(
                                                m * S + wp * 1024 + i * 512,
                                                m * S + wp * 1024
                                                + (i + 1) * 512)
                                            if m % 2 == 0:
                                                nc.vector.tensor_scalar(
                                                    qkT[:, sl], pqs[i][:],
                                                    SM_SCALE,
                                                    bqk_sb[:, m:m + 1],
                                                    op0=MUL, op1=ADD)
                                            else:
                                                nc.vector.tensor_scalar_add(
                                                    qkT[:, sl], pqs[i][:],
                                                    bqk_sb[:, m:m + 1])
                                        if prev is not None:
                                            rope(*prev)
                                    return f
                                yield fin()
                                pending = (m, wp)

                        def last(prev=pending):
                            def f():
                                rope(*prev)
                            return f
                        yield last()

                    def drain(it, n):
                        if it is None:
                            return
                        for _ in range(n):
                            step = next(it, None)
                            if step is None:
                                return
                            step()

                    # ---- V phase (with qk(0) interleaved) ----
                    it0 = qk_steps(0)
                    for tt in range(KC):       # 16 token chunks of 128
                        w, c = divmod(tt, 4)
                        pv = ps.tile([128, 1024], F32, tag="pS", bufs=2,
                                     name=f"pv{tt}")
                        for kc in range(KC):
                            nc.tensor.matmul(
                                pv[:, 0:512],
                                ht_sb[:, w * KC * WIN + kc * WIN + c * 128:
                                      w * KC * WIN + kc * WIN + (c + 1) * 128],
                                wv_sb[:, kc * 512:(kc + 1) * 512],
                                start=(kc == 0), stop=(kc == KC - 1))
                        nc.scalar.copy(v_sb[:, tt * 512:(tt + 1) * 512],
                                       pv[:, 0:512])
                        if tt >= 3:
                            drain(it0, 2 if tt < 8 else 5)
                    drain(it0, 10 ** 6)

                    # wv is dead now; load W_dense rows into its space
                    sv.close()
                    dp = s1.enter_context(tc.tile_pool(name="dp", bufs=1))
                    wd_sb = dp.tile([128, HPC * D], BF16, tag="wd")
                    nc.sync.dma_start(wd_sb[:], wd)

                    # ---- attention for head h, interleaved with filler ----
                    def attn_head(h, filler, pace=None):
                        qb, kb = (2 * h) * S, (2 * h + 1) * S
                        for qs in range(2):        # q blocks of 1024
                            # dual accumulator chains: Pool (even k2) + DVE
                            # (odd k2), both bf16 -- keeps each serial chain
                            # short and off the critical engines
                            ace = glob.tile([128, 1024], BF16, tag="ace",
                                            bufs=2, name=f"ace{h}_{qs}")
                            aco = glob.tile([128, 1024], BF16, tag="aco",
                                            bufs=2, name=f"aco{h}_{qs}")
                            po = ps.tile([128, 1024], F32, tag="po", bufs=1,
                                         name=f"po{h}_{qs}")

                            def consume(pv, po=po, ace=ace, aco=aco, h=h):
                                # Pool sums the first half, DVE the second:
                                # the slow Pool chain finishes mid-block so
                                # the pden matmul never waits on it
                                k2, e = pv
                                for i in range(2):
                                    nc.tensor.matmul(
                                        po[:, i * 512:(i + 1) * 512],
                                        v_sb[:, k2 * 512 + h * 128:
                                             k2 * 512 + (h + 1) * 128],
                                        e[:, i * 512:(i + 1) * 512],
                                        start=(k2 == 0), stop=(k2 == KC - 1))
                                # Pool: even k2 < 12 (one add per 2 iters so
                                # its slow chain keeps up); DVE: the rest,
                                # including the last 4, so the chain tails
                                # are done when pden needs them
                                if k2 == 0:
                                    nc.gpsimd.tensor_copy(ace[:], e[:])
                                elif k2 == 1:
                                    nc.vector.tensor_copy(aco[:], e[:])
                                elif k2 % 2 == 0 and k2 < 12:
                                    nc.gpsimd.tensor_tensor(
                                        ace[:], ace[:], e[:], op=ADD)
                                else:
                                    nc.vector.tensor_tensor(
                                        aco[:], aco[:], e[:], op=ADD)

                            prev = None
                            for k2 in range(KC):   # 16 k chunks of 128
                                pS = ps.tile([128, 1024], F32, tag="pS",
                                             bufs=2, name=f"pS{h}_{qs}_{k2}")
                                for i in range(2):
                                    nc.tensor.matmul(
                                        pS[:, i * 512:(i + 1) * 512],
                                        qkT[:, kb + k2 * 128:kb + (k2 + 1) * 128],
                                        qkT[:, qb + qs * 1024 + i * 512:
                                            qb + qs * 1024 + (i + 1) * 512],
                                        start=True, stop=True)
                                e = epool.tile([128, 1024], BF16, tag="e",
                                               bufs=4, name=f"e{h}_{qs}_{k2}")
                                nc.scalar.activation(e[:], pS[:], EXP)
                                # drain BEFORE consume: the filler's DVE ops
                                # (fins / pd copies) then sit ahead of the
                                # exp-gated aco add in the DVE FIFO, so they
                                # complete at PE pace instead of exp pace
                                n = (pace(qs, k2) if pace is not None
                                     else (2 if k2 < 4 else 1))
                                if n:
                                    drain(filler, n)
                                if prev is not None:
                                    consume(prev)
                                if k2 == 12:
                                    # ace (Pool chain) is complete after its
                                    # k2=10 add; folding it into aco here --
                                    # mid-loop, off the critical path -- lets
                                    # pden launch straight off aco15 at the
                                    # block end
                                    nc.vector.tensor_tensor(
                                        aco[:], aco[:], ace[:], op=ADD)
                                prev = (k2, e)
                            consume(prev)
                            # copy-first normalize: move po to SBUF (oT slice,
                            # unnormalized) on ACT+Pool immediately, freeing
                            # the po PSUM bank for the next q-block's AV chain
                            # before the pden/reciprocal tail completes
                            osl = slice(h * S + qs * 1024,
                                        h * S + (qs + 1) * 1024)
                            osl0 = slice(h * S + qs * 1024,
                                         h * S + qs * 1024 + 512)
                            osl1 = slice(h * S + qs * 1024 + 512,
                                         h * S + (qs + 1) * 1024)
                            # GPSIMD can't read PSUM; one half each on ACT
                            # (ahead of the next block's exp0) and DVE frees
                            # po before the next block's first AV needs it
                            nc.scalar.copy(oT[:, osl0], po[:, 0:512])
                            nc.vector.tensor_copy(oT[:, osl1], po[:, 512:1024])
                            # PE chews filler while the DVE chain tail lands,
                            # so pden never stalls the queue head
                            n = pace(qs, KC) if pace is not None else 2
                            if n:
                                drain(filler, n)
                            pden = ps.tile([128, 1024], F32, tag="pS", bufs=2,
                                           name=f"pden{h}_{qs}")
                            for i in range(2):
                                nc.tensor.matmul(
                                    pden[:, i * 512:(i + 1) * 512],
                                    ones_sb[:],
                                    aco[:, i * 512:(i + 1) * 512],
                                    start=True, stop=True)
                            rcB = epool.tile([128, 1024], BF16, tag="rcB",
                                             bufs=1, name=f"rc{h}_{qs}")
                            with nc.allow_low_precision(
                                    reason="softmax denom reciprocal, bf16 ok"):
                                nc.vector.reciprocal(rcB[:], pden[:])
                            nc.vector.tensor_tensor(
                                oT[:, osl], oT[:, osl], rcB[:], op=MUL)

                    # ---- dense: out[tok, d] = sum_hc oT_hc^T @ wd_hc ----
                    # as a generator so the first tiles can fill head 3's
                    # attention (its qk filler budget is exhausted by then).
                    # Tiles ride the (dead) qk ring as 2x[128,512] and/or the
                    # pS ring as [128,1024]; the post-attention phase
                    # alternates the two rings for an effective depth of 4
                    # tiles so the PSUM->SBUF copies never gate the matmuls.
                    def dense_steps(tts, ring, att=False):
                        idx = 0
                        for tt in tts:
                            for ds in range(2):
                                split = ring(idx)
                                idx += 1
                                last = (tt == KC - 1 and ds == 1)
                                if last:
                                    # independent ring tiles so unit A's copy
                                    # can't be serialized against unit B's MMs
                                    split = True
                                if split:
                                    pds = (ps.tile([128, 512], F32, tag="qk",
                                                   bufs=2,
                                                   name=f"pd{tt}_{ds}a"),
                                           ps.tile([128, 512], F32, tag="qk",
                                                   bufs=2,
                                                   name=f"pd{tt}_{ds}b"))
                                else:
                                    pd = ps.tile([128, 1024], F32, tag="pS",
                                                 bufs=2,
                                                 name=f"pd{tt}_{ds}")
                                    pds = (pd[:, 0:512], pd[:, 512:1024])

                                def mk(hc, tt=tt, ds=ds, pds=pds, iis=(0, 1)):
                                    def f():
                                        for i in iis:
                                            nc.tensor.matmul(
                                                pds[i][:],
                                                oT[:, hc * S + tt * 128:
                                                    hc * S + (tt + 1) * 128],
                                                wd_sb[:, hc * D + ds * 1024
                                                      + i * 512:
                                                      hc * D + ds * 1024
                                                      + (i + 1) * 512],
                                                start=(hc == 0),
                                                stop=(hc == HPC - 1))
                                    return f

                                if last:
                                    # the final tile runs as two serial 512
                                    # units: after the very last MM only one
                                    # 512 copy + one DMA remain before the
                                    # teardown
                                    bt = epool.tile([128, 1024], BF16,
                                                    tag="e", bufs=4,
                                                    name=f"bt{tt}_{ds}")
                                    r = slice(tt * 128, (tt + 1) * 128)
                                    for hc in range(HPC):
                                        yield mk(hc, iis=(0,))

                                    def finA(bt=bt, pds=pds, r=r, ds=ds):
                                        def f():
                                            nc.scalar.copy(bt[:, 0:512],
                                                           pds[0][:])
                                            nc.sync.dma_start(
                                                outp[r, ds * 1024:
                                                     ds * 1024 + 512],
                                                bt[:, 0:512])
                                        return f
                                    yield finA()
                                    for hc in range(HPC):
                                        yield mk(hc, iis=(1,))

                                    def finB(bt=bt, pds=pds, r=r, ds=ds):
                                        def f():
                                            nc.vector.tensor_copy(
                                                bt[:, 512:1024], pds[1][:])
                                            nc.scalar.dma_start(
                                                outp[r, ds * 1024 + 512:
                                                     (ds + 1) * 1024],
                                                bt[:, 512:1024])
                                        return f
                                    yield finB()
                                    continue

                                for hc in range(HPC):
                                    yield mk(hc)

                                def fin(tt=tt, ds=ds, pds=pds):
                                    def f():
                                        bt = epool.tile([128, 1024], BF16,
                                                        tag="e", bufs=4,
                                                        name=f"bt{tt}_{ds}")
                                        i2 = tt * 2 + ds
                                        r = slice(tt * 128, (tt + 1) * 128)
                                        if tt == KC - 1 and ds == 1:
                                            # very last tile: quarter copies
                                            # (ACT+DVE in parallel) first,
                                            # then all DMAs -- a DMA descr.
                                            # on the scalar queue would block
                                            # ACT copies issued after it
                                            for qq in range(4):
                                                bq = slice(qq * 256,
                                                           (qq + 1) * 256)
                                                pq = pds[qq // 2][:, (qq % 2)
                                                                  * 256:
                                                                  (qq % 2)
                                                                  * 256 + 256]
                                                if qq % 2 == 0:
                                                    nc.scalar.copy(
                                                        bt[:, bq], pq)
                                                else:
                                                    nc.vector.tensor_copy(
                                                        bt[:, bq], pq)
                                            for qq in range(4):
                                                bq = slice(qq * 256,
                                                           (qq + 1) * 256)
                                                oq = slice(ds * 1024 + qq * 256,
                                                           ds * 1024
                                                           + (qq + 1) * 256)
                                                q = (nc.sync if qq % 2 == 0
                                                     else nc.scalar)
                                                q.dma_start(outp[r, oq],
                                                            bt[:, bq])
                                            return
                                        if att:
                                            # both halves on DVE: ACT is deep
                                            # with exps during head-3 filler
                                            nc.vector.tensor_copy(
                                                bt[:, 0:512], pds[0][:])
                                            nc.vector.tensor_copy(
                                                bt[:, 512:1024], pds[1][:])
                                        else:
                                            # one half per engine, every tile
                                            nc.scalar.copy(bt[:, 0:512],
                                                           pds[0][:])
                                            nc.vector.tensor_copy(
                                                bt[:, 512:1024], pds[1][:])
                                        if tt == KC - 1:
                                            nc.sync.dma_start(
                                                outp[r, 0:512], bt[:, 0:512])
                                            nc.scalar.dma_start(
                                                outp[r, 512:1024],
                                                bt[:, 512:1024])
                                            return
                                        q = (nc.sync if i2 % 2 == 0
                                             else nc.scalar)
                                        q.dma_start(
                                            outp[r, ds * 1024:(ds + 1) * 1024],
                                            bt[:])
                                    return f
                                yield fin()

                    for h in range(HPC):
                        if h + 1 < HPC:
                            filler, pace = qk_steps(h + 1), None
                        else:
                            # qs0 tail: exactly 3 drains (hc 0-2 of the first
                            # tile -- their oT slices belong to heads 0-2 and
                            # are long ready); hc=3 must wait for the qs0
                            # normalize, safe from qs1 k2>=5
                            filler = dense_steps((0, 1, 2),
                                                 lambda i: True, att=True)
                            pace = (lambda qs, k2:
                                    1 if (qs == 0 and 13 <= k2 < KC)
                                    else (3 if (qs == 1 and k2 >= 5) else 0))
                        attn_head(h, filler, pace)
                        drain(filler, 10 ** 6)
                    drain(dense_steps(range(3, KC), lambda i: i % 2 == 0),
                          10 ** 6)
                if probe is not None:
                    nc.sync.dma_start(probe, bqk_sb[:, 0:4])
    nc.compile()
    return nc


# q/k head-dim permutation: rot_lo -> partitions 0-15, rot_hi -> 32-47
# (32-aligned bases for the DVE rotate_half), pass dims fill the rest.
# scores contract over all 128 dims, so any fixed permutation is exact.
QK_PERM = np.array(list(range(0, 16)) + list(range(32, 48))
                   + list(range(16, 32)) + list(range(48, 128)))


def _rope_tables(position_ids_b):
    pos = np.asarray(position_ids_b, dtype=np.float64)
    inv_freq = 1.0 / (BASE ** (np.arange(0, ROT, 2, dtype=np.float64) / ROT))
    freqs = np.outer(pos, inv_freq)                       # [S, 16]
    cos = np.cos(freqs).T                                 # [16, S] (dup halves)
    sin = np.sin(freqs).T
    cos48 = np.ones((48, cos.shape[1]), np.float64)
    cos48[0:16] = cos          # rot_lo (partitions 0-15)
    cos48[32:48] = cos         # rot_hi (partitions 32-47); 16-31 pass -> x1
    sin48 = np.zeros((48, sin.shape[1]), np.float64)
    sin48[0:16] = sin
    sin48[32:48] = sin
    return (np.ascontiguousarray(cos48).astype(NBF),
            np.ascontiguousarray(sin48).astype(NBF))


def kernel(hidden_states, position_ids, W_qkv, b_qkv, W_dense, b_dense):
    global _NC
    if _NC is None:
        _NC = _build()
    nc = _NC

    hidden_states = np.asarray(hidden_states, dtype=np.float32)
    W_qkv = np.asarray(W_qkv, dtype=np.float32)
    b_qkv = np.asarray(b_qkv, dtype=np.float32)
    W_dense = np.asarray(W_dense, dtype=np.float32)
    b_dense = np.asarray(b_dense, dtype=np.float32)

    ones = np.ones((128, 128), np.float32).astype(NBF)

    # W_qkv columns in NeoX layout: [D, H, 3, HS]
    Wq4 = W_qkv.reshape(D, H, 3, HS)
    bq3 = b_qkv.reshape(H, 3, HS)

    # v-bias contribution is exact post-softmax: attn rows sum to 1
    b_v_full = bq3[:, 2, :].reshape(H * HS)
    b_dense_eff = (b_v_full.astype(np.float64) @ W_dense.astype(np.float64)
                   + b_dense)

    # hT arranged [128, w*KC*WIN + kc*WIN + j]
    hts = []
    for b in range(B):
        hT = np.ascontiguousarray(hidden_states[b].T)        # [D, S]
        h_r = (hT.reshape(KC, 128, NW, WIN).transpose(1, 2, 0, 3)
               .reshape(128, NW * KC * WIN))
        hts.append(np.ascontiguousarray(h_r).astype(NBF))
    tabs_per_b = [_rope_tables(np.asarray(position_ids)[b]) for b in range(B)]

    in_maps = []
    for c in range(NCORES):
        b, g = divmod(c, CPB)
        heads = list(range(g * HPC, (g + 1) * HPC))
        wqk_blocks = []
        bqk_cols = []
        for hgl in heads:
            for part in range(2):                            # 0: q, 1: k
                wm = Wq4[:, hgl, part, :][:, QK_PERM]        # [D, 128]
                wqk_blocks.append(
                    wm.reshape(KC, 128, 128).transpose(1, 0, 2)
                    .reshape(128, KC * 128))
                bias = bq3[hgl, part, QK_PERM].copy()
                if part == 0:
                    bias *= SM_SCALE
                bqk_cols.append(bias)
        wqk_arr = np.concatenate(wqk_blocks, axis=1).astype(NBF)
        bqk_arr = np.stack(bqk_cols, axis=1).astype(np.float32)  # [128, 8]

        wv_g = Wq4[:, heads, 2, :].reshape(D, HPC * HS)          # [D, 512]
        wv_arr = (wv_g.reshape(KC, 128, 512).transpose(1, 0, 2)
                  .reshape(128, KC * 512)).astype(NBF)

        wd_g = W_dense[g * HPC * HS:(g + 1) * HPC * HS, :]       # [512, D]
        wd_arr = (wd_g.reshape(HPC, 128, D).transpose(1, 0, 2)
                  .reshape(128, HPC * D)).astype(NBF)

        cosT, sinT = tabs_per_b[b]
        in_maps.append({
            "ht": hts[b],
            "wqk": np.ascontiguousarray(wqk_arr),
            "wv": np.ascontiguousarray(wv_arr),
            "wd": np.ascontiguousarray(wd_arr),
            "tabc": cosT,
            "tabs": sinT,
            "ones": ones,
            "bqk": np.ascontiguousarray(bqk_arr),
        })

    res = bass_utils.run_bass_kernel_spmd(
        nc, in_maps, core_ids=list(range(NCORES)), trace=TRACE)
    LAST_RESULT[0] = res

    out = np.empty((B, S, D), np.float32)
    for b in range(B):
        acc = np.zeros((S, D), np.float64)
        for g in range(CPB):
            acc += np.asarray(res.results[b * CPB + g]["outp"],
                              dtype=np.float64)
        out[b] = (acc + b_dense_eff).astype(np.float32)
    return out

